# revision 24
# baseline (speedup 1.0000x reference)
"""Fused bmm + residual kernel for Trainium2 (8 NeuronCores, data-parallel).

out[n,c,p] = x[n,c,p] + alpha * sum_q attn[n,p,q] * D[n,q,c]
  N=2048, C=512, H=W=7 (HW=49)

Sharding: batch N across 8 cores (256 each). Each core computes its slice
independently; no collectives.

Per-core scheme (v9: bf16, pair-packed K, block-diagonal rhs, blob I/O):
 - inputs host-packed per group of 16 batches into one [128, 8256] bf16
   blob (x | D | attn^T), one ~2MB DMA per group with 16.5KB contiguous
   runs spread over all 16 SDMA engines.
 - K packs a PAIR of consecutive batches: row k = b*64 + q holds
   D[2i+b, q, :] (q padded to 64, channels permuted c' = j*64+m with
   c = 8m+j) and the block-diagonal rhs rows: attn^T[q, p] of batch
   2i+b in column-block b, zeros elsewhere -- all packed on host, so
   one 64-col weight load serves both batches (halves LDWEIGHTS).
 - per pair: 8 matmuls (j chunks) with K=128, M=64, N=128 (2 batches x
   64 p-slots); output col-groups alternate {0, 64} with j parity.
 - x/out partition k = (j%2)*64 + m holds channels c = 8m + j.
 - alpha applied in the epilogue: one DVE scalar_tensor_tensor per pair
   computes o = y*alpha + x straight from PSUM; out written bf16 in
   tile layout (host unshuffles + upcasts), halving write traffic.
"""
import sys

sys.path.insert(0, "/opt/trn_rl_repo")

import numpy as np

# ---- static problem config (hardcoded per harness contract) ----
N_TOT, C, HW = 2048, 512, 49
QP = 64                      # q/p padded to 64
N_CORES = 8
NB = N_TOT // N_CORES        # 256 batches per core
G = 16                       # batches per group (one DMA round)
NPAIR = G // 2               # pairs per group
NGROUP = NB // G             # groups per core

XL = NPAIR * 8 * HW          # 3136 x elems per partition per group
DL = NPAIR * C               # 4096 d elems
AL = NPAIR * 128             # 1024 attn elems (block-diag 2x49 tight, 128-slot)
BL = XL + DL + AL            # 8256 blob elems per partition per group

_cached = {}


def _build_bass():
    import concourse.bacc as bacc
    import concourse.mybir as mybir
    from concourse import tile
    from concourse.alu_op_type import AluOpType

    f32 = mybir.dt.float32
    bf16 = mybir.dt.bfloat16
    nc = bacc.Bacc(None, target_bir_lowering=False)

    in_d = nc.dram_tensor("blob", [NGROUP, 128, BL], bf16, kind="ExternalInput")
    al_d = nc.dram_tensor("alphac", [128, 1], f32, kind="ExternalInput")
    # output stays in tile layout (bf16); host unshuffles + upcasts
    o_d = nc.dram_tensor(
        "out", [NGROUP, 128, NPAIR * 8 * HW], bf16, kind="ExternalOutput"
    )

    with tile.TileContext(nc) as tc:
        with (
            tc.tile_pool(name="const", bufs=1) as const,
            tc.tile_pool(name="inp", bufs=4) as inp,
            tc.tile_pool(name="op", bufs=4) as op,
            tc.tile_pool(name="yp", bufs=4, space="PSUM") as yp,
        ):
            alpha_sb = const.tile([128, 1], f32)
            nc.sync.dma_start(out=alpha_sb, in_=al_d[:])

            for g in range(NGROUP):
                in_t = inp.tile([128, BL], bf16, tag="in")
                nc.sync.dma_start(out=in_t, in_=in_d[g])

                # views into the blob
                x_v = in_t[:, 0:XL].rearrange("k (i f) -> k i f", i=NPAIR)
                d_v = in_t[:, XL:XL + DL].rearrange("k (i c) -> k i c", i=NPAIR)
                a_v = in_t[:, XL + DL:BL].rearrange("k (i n) -> k i n", i=NPAIR)

                o_t = op.tile([128, NPAIR, 8 * HW], bf16, tag="o")

                for i2 in range(NPAIR // 2):
                    # 2 pairs share one 2-bank PSUM tile -> one epilogue op
                    # u-stride = 512 f32 = one full PSUM bank, so every
                    # matmul slice stays inside a single bank
                    y_ps = yp.tile([128, 2, 512], f32, tag="y")
                    for u in range(2):
                        i = 2 * i2 + u
                        for j in range(8):
                            # one weight load serves both batches (rhs is
                            # block-diagonal); col-group alternates j%2
                            jp = j % 2
                            t = j // 2
                            nc.tensor.matmul(
                                out=y_ps[
                                    jp * 64:(jp + 1) * 64,
                                    u,
                                    t * 2 * HW:(t + 1) * 2 * HW,
                                ],
                                lhsT=d_v[:, i, j * 64:(j + 1) * 64],
                                rhs=a_v[:, i, 0:2 * HW],
                                start=True,
                                stop=True,
                            )
                    # o = y*alpha + x for both pairs, straight from PSUM
                    nc.vector.scalar_tensor_tensor(
                        out=o_t[:, 2 * i2:2 * i2 + 2, :],
                        in0=y_ps[:, :, 0:8 * HW],
                        scalar=alpha_sb,
                        in1=x_v[:, 2 * i2:2 * i2 + 2, :],
                        op0=AluOpType.mult,
                        op1=AluOpType.add,
                    )
                    # half-group stores in tile layout (3.1KB runs);
                    # alternate ACT (HWDGE) / GpSimd (SWDGE) so the load
                    # ring never blocks and descriptor gen parallelizes
                    if i2 == 1:
                        nc.scalar.dma_start(
                            out=o_d[g, :, 0:4 * 8 * HW], in_=o_t[:, 0:4, :]
                        )
                    elif i2 == 3:
                        nc.gpsimd.dma_start(
                            out=o_d[g, :, 4 * 8 * HW:], in_=o_t[:, 4:8, :]
                        )

    nc.finalize()
    return nc


def _get_nc():
    if "nc" not in _cached:
        _cached["nc"] = _build_bass()
    return _cached["nc"]


def _in_maps(x, attn, D, alpha):
    import ml_dtypes

    bf16 = np.dtype(ml_dtypes.bfloat16)
    Nb, Ng = N_CORES * NGROUP, NPAIR  # flatten (core, group) for packing
    # x part: partition k = (j%2)*64 + m holds c = 8m + j = 8m + 2t + jp
    xb = (
        np.asarray(x, np.float32)
        .reshape(Nb, Ng, 2, 64, 4, 2, HW)   # (cg, i, b, m, t, jp, p)
        .transpose(0, 5, 3, 1, 4, 2, 6)     # (cg, jp, m, i, t, b, p)
        .astype(bf16)
        .reshape(Nb, 128, XL)
    )
    # d part: row k = b*64+q^ (q padded), channels c' = j*64 + m (c = 8m+j)
    perm = (np.arange(C // 8)[None, :] * 8 + np.arange(8)[:, None]).ravel()
    db = np.zeros((Nb, 2, QP, Ng, C), bf16)  # (cg, b, q^, i, c')
    db[:, :, :HW, :, :] = (
        np.asarray(D, np.float32)[:, :, perm]
        .reshape(Nb, Ng, 2, HW, C)           # (cg, i, b, q, c')
        .transpose(0, 2, 3, 1, 4)            # (cg, b, q, i, c')
        .astype(bf16)
    )
    db = db.reshape(Nb, 128, DL)
    # attn part, block-diagonal: row k = b*64+q^, col-block b' = attn^T
    # of batch 2i+b if b'==b else 0; p in 64-slots
    at = (
        np.asarray(attn, np.float32)
        .reshape(Nb, Ng, 2, HW, HW)          # (cg, i, b, p, q)
        .transpose(0, 2, 4, 1, 3)            # (cg, b, q, i, p)
        .astype(bf16)
    )
    ab = np.zeros((Nb, 2, QP, Ng, 128), bf16)  # (cg, b, q^, i, b'*49+p)
    for b in range(2):
        ab[:, b, :HW, :, b * HW:(b + 1) * HW] = at[:, b]
    ab = ab.reshape(Nb, 128, AL)
    blob = np.concatenate([xb, db, ab], axis=2).reshape(
        N_CORES, NGROUP, 128, BL
    )
    al = np.full((128, 1), np.float32(np.asarray(alpha).reshape(-1)[0]), np.float32)
    return [{"blob": blob[c], "alphac": al} for c in range(N_CORES)]


def kernel(x: np.ndarray, attn: np.ndarray, D: np.ndarray, alpha: np.ndarray) -> np.ndarray:
    from concourse import bass_utils

    nc = _get_nc()
    res = bass_utils.run_bass_kernel_spmd(
        nc, _in_maps(x, attn, D, alpha), core_ids=list(range(N_CORES))
    )
    out = np.stack([res.results[c]["out"] for c in range(N_CORES)])
    # undo tile layout: (cg, jp, m, i, t, b, p) -> (cg, i, b, m, t, jp, p)
    out = (
        out.reshape(N_CORES * NGROUP, 2, 64, NPAIR, 4, 2, HW)
        .transpose(0, 3, 5, 2, 4, 1, 6)
        .astype(np.float32)
    )
    return np.ascontiguousarray(out).reshape(N_TOT, C, 7, 7)
